# revision 6
# baseline (speedup 1.0000x reference)
"""MetaDGCRU Trainium2 kernel.

Problem (hardcoded shapes): B=8, N=400, INPUT_DIM=2, HIDDEN=64,
GRAPH_NUM=2, HOP_K=2, NODE_EMB_DIM=16, IN_FEAT=66, I_DIM=330.

Sharding: data-parallel over batch B across the 8 NeuronCores (one batch
element per core); weight pools replicated, per-graph adjacencies sharded
with their batch.

Per-core computation (feature-on-partition / "transposed" layouts):
  xsT = [x;state].T                                    [66, 400]
  hops transposed-out:  YT = lhsT(X_nat).T @ AT        (PE, 4 m-chunks)
  hT = concat pieces -> 3 tiles of [128, 400] (i padded 330->384)
  gT[(d,i), n] = embT[d,n] * hT[i,n]                   (DVE, (k,q) wavefront)
  zrT = bias(start=True, K=16) + sum_c Wg[c].T @ gT[c] (PE, 48 + 1 MMs)
  z,r = sigmoid(zrT);  xrsT = [xT; rT*stateT];  repeat -> hcT = tanh(...)
  out hT = hcT + (1-z)*... = hc + omz*hc' blend        [64, 400] f32

DMA strategy (v2): every dma_start costs ~0.65us of blocking sequencer
time (DIRECT2D), so inputs are consolidated into few fat triggers split
across BOTH HWDGE rings in consumption order:
  SP ring:  adj g0, adj g1, wg x4 (group-packed), wc x2
  ACT ring: smalls-pack, xsT, embrep q0..q3 (q2/q3 interleaved after the
            gate y1 copies so they don't delay the hop-chain ACT work)
  GpSimd SWDGE ring: the 8 mid-kernel piece DMAs (cross-partition moves)
The gT build/matmul groups run in a (k,q) wavefront so k=0 groups overlap
the hop chain (k0 needs only hT tile0 = xs + y1g0) and late groups tolerate
late embrep/wg arrival.
"""

import os

os.environ.setdefault("MYCRO_LOCAL_CACHE", "1")

import numpy as np
import ml_dtypes

B, N = 8, 400
INPUT_DIM, HIDDEN = 2, 64
GRAPH_NUM, HOP_K = 2, 2
D_EMB = 16
IN_FEAT = INPUT_DIM + HIDDEN               # 66
I_DIM = (GRAPH_NUM * HOP_K + 1) * IN_FEAT  # 330
KCH = 3                                    # i-chunks per d (128 each)
I_PAD = KCH * 128                          # 384
NCH = D_EMB * KCH                          # 48 total K chunks
O_G = 2 * HIDDEN                           # 128 gate out (z|r)
O_C = HIDDEN                               # 64 candidate out
NPAD = 512                                 # node dim padded for clean DMA packing

BF16 = ml_dtypes.bfloat16
MCHUNKS = [(0, 128), (128, 128), (256, 128), (384, 16)]  # node-dim chunking
QD = 4                                     # d's per gT group

# (k, q) groups, q-outer / k-inner: embrep quarter q and wg third (q)
# arrive in stream order while the hT tiles (k) fill fast from the hop
# chain.  12 groups x 4 chunks = 48 chunks.
GROUPS = [(0, 0), (1, 0), (2, 0), (0, 1), (1, 1), (2, 1),
          (0, 2), (1, 2), (2, 2), (0, 3), (1, 3), (2, 3)]

# smalls-pack column layout (one [128, SMALL_W] bf16 HBM tensor)
XSNAT_C = 0                      # [128, 264] xs natural (k f)
IDENT_C = XSNAT_C + 4 * IN_FEAT  # [128, 128] identity
STATE2_C = IDENT_C + 128         # [128, 400] stateT stacked twice (bf16)
EMBT_C = STATE2_C + N            # [16, 400] embT (rows 0:16)
BG_C = EMBT_C + N                # [16, 128] gate bias
BC_C = BG_C + O_G                # [16, 64] cand bias
XROW_C = BC_C + O_C              # [2, 400] x.T (rows 0:2)
SMALL_W = XROW_C + N

_CACHE = {}


def _emit(nc, tc, tile, mybir, ctx):
    """Emit the per-core kernel into TileContext tc."""
    dt = mybir.dt
    Sig = mybir.ActivationFunctionType.Sigmoid
    Tanh = mybir.ActivationFunctionType.Tanh
    Copy = mybir.ActivationFunctionType.Copy

    d_adj = nc.dram_tensor("adj", [GRAPH_NUM, 128, 4 * N], dt.bfloat16, kind="ExternalInput")
    d_smalls = nc.dram_tensor("smalls", [128, SMALL_W], dt.bfloat16, kind="ExternalInput")
    d_xsT = nc.dram_tensor("xsT", [IN_FEAT, N], dt.bfloat16, kind="ExternalInput")
    d_embrep = nc.dram_tensor("embrep", [128, D_EMB * N], dt.bfloat16, kind="ExternalInput")
    d_wg = nc.dram_tensor("wg", [128, NCH * O_G], dt.bfloat16, kind="ExternalInput")
    d_wc = nc.dram_tensor("wc", [128, NCH * O_C], dt.bfloat16, kind="ExternalInput")
    d_out = nc.dram_tensor("out", [HIDDEN, N], dt.float32, kind="ExternalOutput")

    cpool = ctx.enter_context(tc.tile_pool(name="const", bufs=1))
    hpool = ctx.enter_context(tc.tile_pool(name="hbuf", bufs=1))
    gpool = ctx.enter_context(tc.tile_pool(name="gbuf", bufs=1))
    spool = ctx.enter_context(tc.tile_pool(name="small", bufs=4))
    ppool = ctx.enter_context(tc.tile_pool(name="psum", bufs=2, space="PSUM"))
    ptp = ctx.enter_context(tc.tile_pool(name="psumT", bufs=2, space="PSUM"))
    pzr = ctx.enter_context(tc.tile_pool(name="psumZR", bufs=1, space="PSUM"))
    pbc = ctx.enter_context(tc.tile_pool(name="psumBC", bufs=2, space="PSUM"))

    # ---- SP-ring bulk triggers (priority = emission order per ring):
    # adjacency first, then embrep quarters interleaved with wg thirds in
    # group-consumption order, then wc ----
    at_sb = []
    for g in range(GRAPH_NUM):
        t = cpool.tile([128, 4 * N], dt.bfloat16, name=f"adj{g}")
        nc.sync.dma_start(t[:], d_adj[g, :, :])
        at_sb.append(t)
    wg_sb = cpool.tile([128, NCH * O_G], dt.bfloat16, name="wg")
    embrep_sb = cpool.tile([128, D_EMB * N], dt.bfloat16, name="embrep")
    WGT = NCH * O_G // 4
    for q4 in range(4):
        e0 = q4 * QD * N
        nc.sync.dma_start(embrep_sb[:, e0:e0 + QD * N],
                          d_embrep[:, e0:e0 + QD * N])
        nc.sync.dma_start(wg_sb[:, q4 * WGT:(q4 + 1) * WGT],
                          d_wg[:, q4 * WGT:(q4 + 1) * WGT])
    wc_sb = cpool.tile([128, NCH * O_C], dt.bfloat16, name="wc")
    WCT = NCH * O_C // 2
    for t2 in range(2):
        nc.sync.dma_start(wc_sb[:, t2 * WCT:(t2 + 1) * WCT],
                          d_wc[:, t2 * WCT:(t2 + 1) * WCT])

    # ---- ACT-ring: smalls pack + xsT only (keeps the ACT sequencer free
    # for the hop-chain copies) ----
    smalls = cpool.tile([128, SMALL_W], dt.bfloat16, name="smalls")
    nc.scalar.dma_start(smalls[:], d_smalls[:, :])
    xsnat_v = smalls[:, XSNAT_C:XSNAT_C + 4 * IN_FEAT]
    ident_v = smalls[:, IDENT_C:IDENT_C + 128]
    state2_v = smalls[:, STATE2_C:STATE2_C + N]
    embT_v = smalls[0:D_EMB, EMBT_C:EMBT_C + N]
    bg_v = smalls[0:D_EMB, BG_C:BG_C + O_G]
    bc_v = smalls[0:D_EMB, BC_C:BC_C + O_C]
    xrow_v = smalls[0:INPUT_DIM, XROW_C:XROW_C + N]

    # hT tiles + xs loads
    hT_g = [hpool.tile([128, N], dt.bfloat16, name=f"hTg{t}") for t in range(KCH)]
    hT_c = [hpool.tile([128, N], dt.bfloat16, name=f"hTc{t}") for t in range(KCH)]
    nc.vector.memset(hT_g[2][:, :], 0.0)
    nc.vector.memset(hT_c[2][:, :], 0.0)
    nc.scalar.dma_start(hT_g[0][0:IN_FEAT, :], d_xsT[:, :])

    # x rows into candidate hT tile0 (same partitions -> ACT copy, no DMA)
    nc.scalar.activation(hT_c[0][0:INPUT_DIM, :], xrow_v, Copy)

    # dummy matmuls warm the PE (HAM) while the adjacency streams in
    ones_sb = cpool.tile([128, 256], dt.bfloat16, name="ones_sb")
    nc.vector.memset(ones_sb[:, :], 1.0)
    for w in range(6):
        warm_ps = pbc.tile([128, 192], dt.float32, name=f"warm_ps{w}", tag="warmps", bufs=1)
        nc.tensor.matmul(warm_ps[:], ones_sb[:, 0:128], ones_sb[:, 0:192],
                         start=True, stop=True)

    # warm the ACT Copy table early (first pieceT copy needs it)
    warm = hpool.tile([1, 8], dt.float32, name="warm")
    nc.vector.memset(warm[:, :], 0.0)
    nc.scalar.activation(warm[:, 0:4], warm[:, 4:8], Copy)

    # gT buffer: 48 chunks of [128, N] in GROUP order (shared gate/cand)
    gT = gpool.tile([128, NCH * N], dt.bfloat16, name="gT")

    def piece_to_hT(hT, piece, piece_ps, p_idx, cand=False):
        """Place piece [IN_FEAT, N] into hT tiles. Split pieces (1 and 3)
        put their leading spill rows in the next tile via a base-0 ACT copy
        straight from PSUM (the host W-pack permutation compensates); the
        main part goes via a GpSimd SWDGE DMA (idle third ring). The
        candidate layout moves piece 1's main part to tile0[2:64]."""
        if p_idx == 1:
            nc.scalar.activation(hT[1][0:4, :], piece_ps[0:4, :], Copy)
            dst = hT[0][2:64, :] if cand else hT[0][66:128, :]
            return [nc.gpsimd.dma_start(dst, piece[4:IN_FEAT, :])]
        if p_idx == 3:
            nc.scalar.activation(hT[2][0:8, :], piece_ps[0:8, :], Copy)
            return [nc.gpsimd.dma_start(hT[1][70:128, :], piece[8:IN_FEAT, :])]
        r0 = IN_FEAT * p_idx
        t0, o0 = divmod(r0, 128)
        return [nc.gpsimd.dma_start(hT[t0][o0:o0 + IN_FEAT, :], piece[:, :])]

    def hop(lhsT_of, g, name):
        """One propagation Y = A_g @ X, transposed out. lhsT_of(k)->AP [mlen,66]."""
        yt_ps = ppool.tile([IN_FEAT, N], dt.float32, name=f"ps_{name}", tag="hopps")
        for k, (moff, mlen) in enumerate(MCHUNKS):
            rhs = (at_sb[g][:, k * N:(k + 1) * N] if k < 3
                   else at_sb[g][0:16, 3 * N:4 * N])
            nc.tensor.matmul(
                yt_ps[:], lhsT_of(k), rhs,
                start=(k == 0), stop=(k == len(MCHUNKS) - 1),
            )
        yt = spool.tile([IN_FEAT, N], dt.bfloat16, name=f"yt_{name}", tag="hopsb")
        nc.scalar.activation(yt[:], yt_ps[:], Copy)
        return yt, yt_ps

    def nat_slicer(tl):
        return lambda k: tl[0:MCHUNKS[k][1], k * IN_FEAT:(k + 1) * IN_FEAT]

    def naturalize(yt, name):
        """PE-transpose YT [66, N] -> natural tile [128, 4*66]."""
        natt = spool.tile([128, 4 * IN_FEAT], dt.bfloat16, name=f"nat_{name}", tag="natsb")
        for k, (moff, mlen) in enumerate(MCHUNKS):
            tp = ptp.tile([mlen, IN_FEAT], dt.bfloat16, name=f"tp_{name}{k}", tag="trps")
            nc.tensor.transpose(tp[:], yt[:, moff:moff + mlen], ident_v[0:IN_FEAT, 0:IN_FEAT])
            nc.scalar.activation(natt[0:mlen, k * IN_FEAT:(k + 1) * IN_FEAT], tp[:], Copy)
        return natt

    filler_ctr = [100]

    def pe_fillers(n):
        for _ in range(n):
            warm_ps = pbc.tile([128, 192], dt.float32,
                               name=f"warm_ps{filler_ctr[0]}", tag="warmps", bufs=1)
            filler_ctr[0] += 1
            nc.tensor.matmul(warm_ps[:], ones_sb[:, 0:128], ones_sb[:, 0:192],
                             start=True, stop=True)

    def meta_phase(hT, lhsT_of, w_sb, b_sb, o_dim, psum_out, phase, cand=False):
        """Hops + gT build + meta matmul, accumulating into psum_out [o_dim, N]."""
        # both first hops are independent: run them (and their pieces) first
        y1 = []
        for g in range(GRAPH_NUM):
            y1t, y1ps = hop(lhsT_of, g, f"{phase}y1g{g}")
            piece_to_hT(hT, y1t, y1ps, 1 + 2 * g, cand=cand)
            y1.append(y1t)
        y1nat = [naturalize(y1[g], f"{phase}g{g}") for g in range(GRAPH_NUM)]
        for g in range(GRAPH_NUM):
            y2t, y2ps = hop(nat_slicer(y1nat[g]), g, f"{phase}y2g{g}")
            piece_to_hT(hT, y2t, y2ps, 2 + 2 * g, cand=cand)

        if not cand:
            # load sigma/tanh ACT tables in the phase's ACT slack window
            nc.scalar.activation(warm[:, 0:4], warm[:, 4:8], Sig)
            nc.scalar.activation(warm[:, 0:4], warm[:, 4:8], Tanh)
        # bias matmul resets PSUM
        nc.tensor.matmul(psum_out[:], b_sb[:], embT_v, start=True, stop=False)

        # gT build (fused 4-d DVE ops) + accumulate matmuls; (k,q) wavefront
        for gi, (k, q) in enumerate(GROUPS):
            d0 = q * QD
            out_ap = (gT[:, gi * QD * N:(gi + 1) * QD * N]
                      .rearrange("p (c n) -> p c n", n=N))
            in0 = (hT[k][:, :].rearrange("p (u n) -> p u n", u=1)
                   .broadcast_to([128, QD, N]))
            in1 = (embrep_sb[:, d0 * N:(d0 + QD) * N]
                   .rearrange("p (c n) -> p c n", n=N))
            nc.vector.tensor_tensor(out_ap, in0, in1, mybir.AluOpType.mult)
            for j in range(QD):
                c = gi * QD + j
                nc.tensor.matmul(
                    psum_out[:],
                    w_sb[:, c * o_dim:(c + 1) * o_dim],
                    gT[:, c * N:(c + 1) * N],
                    start=False,
                    stop=(gi == len(GROUPS) - 1 and j == QD - 1),
                )

    # ================= gate phase =================
    zr_ps = pzr.tile([O_G, N], dt.float32, name="zr_ps")
    meta_phase(hT_g, nat_slicer(xsnat_v), wg_sb, bg_v, O_G, zr_ps, "g")
    zr_sig = hpool.tile([O_G, N], dt.float32, name="zr_sig")
    # r-half first so the candidate chain starts as early as possible
    nc.scalar.activation(zr_sig[HIDDEN:O_G, :], zr_ps[HIDDEN:O_G, :], Sig)
    nc.scalar.activation(zr_sig[0:HIDDEN, :], zr_ps[0:HIDDEN, :], Sig)

    # rs written straight into the candidate hT tile (base 64, no shift DMA);
    # the Wc host packing uses the matching i-permutation
    nc.vector.tensor_mul(hT_c[0][HIDDEN:O_G, :], zr_sig[HIDDEN:O_G, :],
                         state2_v[HIDDEN:O_G, :])

    # keep the PE busy across the sigma/rs transition
    pe_fillers(6)
    # xrs natural from the two aligned regions: x rows 0:2, rs rows 64:128
    xrsnat = spool.tile([128, 4 * IN_FEAT], dt.bfloat16, name="nat_xrs", tag="natsb")
    for k, (moff, mlen) in enumerate(MCHUNKS):
        tpx = ptp.tile([mlen, INPUT_DIM], dt.bfloat16, name=f"tpx{k}", tag="trpsx", bufs=1)
        nc.tensor.transpose(tpx[:], hT_c[0][0:INPUT_DIM, moff:moff + mlen],
                            ident_v[0:INPUT_DIM, 0:INPUT_DIM])
        nc.scalar.activation(
            xrsnat[0:mlen, k * IN_FEAT:k * IN_FEAT + INPUT_DIM], tpx[:], Copy)
        tpr = ptp.tile([mlen, HIDDEN], dt.bfloat16, name=f"tpr{k}", tag="trps")
        nc.tensor.transpose(tpr[:], hT_c[0][HIDDEN:O_G, moff:moff + mlen],
                            ident_v[HIDDEN:O_G, HIDDEN:O_G])
        nc.scalar.activation(
            xrsnat[0:mlen, k * IN_FEAT + INPUT_DIM:(k + 1) * IN_FEAT], tpr[:], Copy)

    # ================= candidate phase =================
    hc_ps = pzr.tile([O_C, N], dt.float32, name="hc_ps")
    meta_phase(hT_c, nat_slicer(xrsnat), wc_sb, bc_v, O_C, hc_ps, "c", cand=True)

    # z-dependent blend terms precomputed while the candidate phase runs:
    # h = hc + z*(state - hc) = (1-z)*hc + z*state
    omz = hpool.tile([O_C, N], dt.float32, name="omz")
    nc.vector.tensor_scalar(omz[:], zr_sig[0:HIDDEN, :], -1.0, 1.0,
                            mybir.AluOpType.mult, mybir.AluOpType.add)
    zs = hpool.tile([O_C, N], dt.float32, name="zs")
    nc.vector.tensor_mul(zs[:], zr_sig[0:HIDDEN, :], state2_v[0:HIDDEN, :])

    hc_t = hpool.tile([O_C, N], dt.float32, name="hc_t")
    nc.scalar.activation(hc_t[:], hc_ps[:], Tanh)

    # ================= output blend =================
    d2 = hpool.tile([O_C, N], dt.float32, name="d2")
    nc.vector.tensor_mul(d2[:], omz[:], hc_t[:])
    hout = hpool.tile([O_C, N], dt.float32, name="hout")
    nc.vector.tensor_add(hout[:], d2[:], zs[:])
    nc.sync.dma_start(d_out[:, :], hout[:])


def _build_nc():
    import concourse.tile as tile
    import concourse.mybir as mybir
    from contextlib import ExitStack
    from concourse import bacc

    nc = bacc.Bacc(trn_type="TRN2")
    with tile.TileContext(nc) as tc:
        with ExitStack() as ctx:
            _emit(nc, tc, tile, mybir, ctx)
    nc.finalize()
    return nc


def _prep_core_inputs(b, x, state, graphs, node_emb, Wg, bg, Wc, bc):
    """Host-side shard + layout prep for core b. Layouts match SBUF tiles."""
    f32 = np.float32
    at = graphs[:, b].transpose(0, 2, 1)                         # [G, N, N] = A.T
    at_pk = (at[:, :384, :].reshape(GRAPH_NUM, 3, 128, N)
             .transpose(0, 2, 1, 3)
             .reshape(GRAPH_NUM, 128, 3 * N))                    # [G,128,(k n)]
    at3 = at[:, 384:400, :]                                      # [G,16,N]
    adj = np.zeros((GRAPH_NUM, 128, 4 * N), f32)
    adj[:, :, :3 * N] = at_pk
    adj[:, 0:16, 3 * N:] = at3

    xs = np.concatenate([x[b], state[b]], axis=-1)               # [N, 66] f32
    xsT = np.ascontiguousarray(xs.T).astype(BF16)                # [66, N]
    xs_pad = np.zeros((NPAD, IN_FEAT), f32)
    xs_pad[:N] = xs
    xsnat = (xs_pad.reshape(4, 128, IN_FEAT)
             .transpose(1, 0, 2)
             .reshape(128, 4 * IN_FEAT))                         # [128,(k f)]
    stT = np.ascontiguousarray(state[b].T)                       # [64, N] f32
    embT = np.ascontiguousarray(node_emb[b].T).astype(BF16)      # [16, N]
    embrep = np.ascontiguousarray(np.broadcast_to(
        embT.reshape(1, D_EMB * N), (128, D_EMB * N)))           # [128, 16N]

    smalls = np.zeros((128, SMALL_W), f32)
    smalls[:, XSNAT_C:XSNAT_C + 4 * IN_FEAT] = xsnat
    smalls[:, IDENT_C:IDENT_C + 128] = np.eye(128, dtype=f32)
    smalls[0:HIDDEN, STATE2_C:STATE2_C + N] = stT
    smalls[HIDDEN:O_G, STATE2_C:STATE2_C + N] = stT
    smalls[0:D_EMB, EMBT_C:EMBT_C + N] = embT.astype(f32)
    smalls[0:D_EMB, BG_C:BG_C + O_G] = bg
    smalls[0:D_EMB, BC_C:BC_C + O_C] = bc
    smalls[0:INPUT_DIM, XROW_C:XROW_C + N] = x[b].T

    def pack_w(W, o_dim, perm):
        # W [16, 330, o] -> [128, 48*o] in GROUP chunk order; chunk (d,k):
        # padded row r=128k+p holds reference feature perm[r]
        Wp = np.zeros((D_EMB, I_PAD, o_dim), np.float32)
        valid = perm >= 0
        Wp[:, valid, :] = W[:, perm[valid], :]
        Wp = Wp.reshape(D_EMB, KCH, 128, o_dim)                  # [d,k,p,o]
        cols = np.empty((128, NCH * o_dim), np.float32)
        ci = 0
        for (k, q) in GROUPS:
            for j in range(QD):
                d = q * QD + j
                cols[:, ci * o_dim:(ci + 1) * o_dim] = Wp[d, k]
                ci += 1
        return np.ascontiguousarray(cols).astype(BF16)

    # spill permutation (both phases): pieces 1/3 put their first 4/8 rows
    # in the next tile, so main parts shift by the spill size
    perm_g = np.arange(I_PAD, dtype=np.int64)
    perm_g[I_DIM:] = -1
    perm_g[66:128] = np.arange(70, 132)
    perm_g[128:132] = np.arange(66, 70)
    perm_g[198:256] = np.arange(206, 264)
    perm_g[256:264] = np.arange(198, 206)
    # candidate adds: rows 2:64 <- Y1g0 main (ref 70:132), rows 64:128 <- rs
    perm_c = perm_g.copy()
    perm_c[0:INPUT_DIM] = np.arange(0, INPUT_DIM)
    perm_c[2:64] = np.arange(70, 132)
    perm_c[64:128] = np.arange(2, 66)
    perm_c[128:132] = np.arange(66, 70)

    return {
        "adj": np.ascontiguousarray(adj).astype(BF16),
        "smalls": np.ascontiguousarray(smalls).astype(BF16),
        "xsT": xsT,
        "embrep": embrep,
        "wg": pack_w(Wg, O_G, perm_g),
        "wc": pack_w(Wc, O_C, perm_c),
    }


def kernel_with_results(x, state, graphs, node_emb, Wg, bg, Wc, bc, trace=False):
    from concourse.bass_utils import run_bass_kernel_spmd

    x = np.asarray(x, np.float32)
    state = np.asarray(state, np.float32)
    graphs = np.asarray(graphs, np.float32)
    node_emb = np.asarray(node_emb, np.float32)
    Wg = np.asarray(Wg, np.float32)
    bg = np.asarray(bg, np.float32)
    Wc = np.asarray(Wc, np.float32)
    bc = np.asarray(bc, np.float32)

    if "nc" not in _CACHE:
        _CACHE["nc"] = _build_nc()
    nc = _CACHE["nc"]

    in_maps = [
        _prep_core_inputs(b, x, state, graphs, node_emb, Wg, bg, Wc, bc)
        for b in range(B)
    ]
    res = run_bass_kernel_spmd(nc, in_maps, core_ids=list(range(B)), trace=trace)
    out = np.stack(
        [np.ascontiguousarray(res.results[b]["out"].T) for b in range(B)], axis=0
    )  # [B, N, HIDDEN] f32
    return out, res


def kernel(**inputs):
    out, _ = kernel_with_results(**inputs)
    return out


# revision 14
# speedup vs baseline: 1.0797x; 1.0797x over previous
"""MetaDGCRU Trainium2 kernel.

Problem (hardcoded shapes): B=8, N=400, INPUT_DIM=2, HIDDEN=64,
GRAPH_NUM=2, HOP_K=2, NODE_EMB_DIM=16, IN_FEAT=66, I_DIM=330.

Sharding: data-parallel over batch B across the 8 NeuronCores (one batch
element per core); weight pools replicated, per-graph adjacencies sharded
with their batch.

Per-core computation (feature-on-partition / "transposed" layouts):
  xsT = [x;state].T                                    [66, 400]
  hops transposed-out:  YT = lhsT(X_nat).T @ AT        (PE, 4 m-chunks)
  hT = concat pieces -> 3 tiles of [128, 400] (i padded 330->384)
  gT[(d,i), n] = embT[d,n] * hT[i,n]                   (DVE, (k,q) wavefront)
  zrT = bias(start=True, K=16) + sum_c Wg[c].T @ gT[c] (PE, 48 + 1 MMs)
  z,r = sigmoid(zrT);  xrsT = [xT; rT*stateT];  repeat -> hcT = tanh(...)
  out hT = hcT + (1-z)*... = hc + omz*hc' blend        [64, 400] f32

DMA strategy (v2): every dma_start costs ~0.65us of blocking sequencer
time (DIRECT2D), so inputs are consolidated into few fat triggers split
across BOTH HWDGE rings in consumption order:
  SP ring:  adj g0, adj g1, wg x4 (group-packed), wc x2
  ACT ring: smalls-pack, xsT, embrep q0..q3 (q2/q3 interleaved after the
            gate y1 copies so they don't delay the hop-chain ACT work)
  GpSimd SWDGE ring: the 8 mid-kernel piece DMAs (cross-partition moves)
The gT build/matmul groups run in a (k,q) wavefront so k=0 groups overlap
the hop chain (k0 needs only hT tile0 = xs + y1g0) and late groups tolerate
late embrep/wg arrival.
"""

import os

os.environ.setdefault("MYCRO_LOCAL_CACHE", "1")

import numpy as np
import ml_dtypes

B, N = 8, 400
INPUT_DIM, HIDDEN = 2, 64
GRAPH_NUM, HOP_K = 2, 2
D_EMB = 16
IN_FEAT = INPUT_DIM + HIDDEN               # 66
I_DIM = (GRAPH_NUM * HOP_K + 1) * IN_FEAT  # 330
KCH = 3                                    # i-chunks per d (128 each)
I_PAD = KCH * 128                          # 384
NCH = D_EMB * KCH                          # 48 total K chunks
O_G = 2 * HIDDEN                           # 128 gate out (z|r)
O_C = HIDDEN                               # 64 candidate out
NPAD = 512                                 # node dim padded for clean DMA packing

BF16 = ml_dtypes.bfloat16
MCHUNKS = [(0, 128), (128, 128), (256, 128), (384, 16)]  # node-dim chunking
QD = 4                                     # d's per gT group

# (k, q) groups, k-outer / q-inner: the DVE queue is FIFO, so groups must
# be ordered by dependency readiness -- hT tile k fills in k order from the
# hop chain (k2 last), while embrep quarters all arrive early in the
# stream.  12 groups x 4 chunks = 48 chunks.
GROUPS = [(0, 0), (0, 1), (0, 2), (0, 3), (1, 0), (1, 1),
          (1, 2), (1, 3), (2, 0), (2, 1), (2, 2), (2, 3)]

# smalls-pack column layout (one [128, SMALL_W] bf16 HBM tensor)
XSNAT_C = 0                      # [128, 264] xs natural (k f)
IDENT_C = XSNAT_C + 4 * IN_FEAT  # [128, 128] identity
STATE2_C = IDENT_C + 128         # [128, 400] stateT stacked twice (bf16)
EMBT_C = STATE2_C + N            # [16, 400] embT (rows 0:16)
BG_C = EMBT_C + N                # [16, 128] gate bias
BC_C = BG_C + O_G                # [16, 64] cand bias
XROW_C = BC_C + O_C              # [2, 400] x.T (rows 0:2)
SMALL_W = XROW_C + N

_CACHE = {}


def _emit(nc, tc, tile, mybir, ctx):
    """Emit the per-core kernel into TileContext tc."""
    dt = mybir.dt
    Sig = mybir.ActivationFunctionType.Sigmoid
    Tanh = mybir.ActivationFunctionType.Tanh
    Copy = mybir.ActivationFunctionType.Copy

    d_adj = nc.dram_tensor("adj", [GRAPH_NUM, 128, 4 * N], dt.bfloat16, kind="ExternalInput")
    d_smalls = nc.dram_tensor("smalls", [128, SMALL_W], dt.bfloat16, kind="ExternalInput")
    d_xsT = nc.dram_tensor("xsT", [IN_FEAT, N], dt.bfloat16, kind="ExternalInput")
    d_embrep = nc.dram_tensor("embrep", [128, D_EMB * N], dt.bfloat16, kind="ExternalInput")
    d_wg = nc.dram_tensor("wg", [128, NCH * O_G], dt.bfloat16, kind="ExternalInput")
    d_wc = nc.dram_tensor("wc", [128, NCH * O_C], dt.bfloat16, kind="ExternalInput")
    d_out = nc.dram_tensor("out", [HIDDEN, N], dt.float32, kind="ExternalOutput")

    cpool = ctx.enter_context(tc.tile_pool(name="const", bufs=1))
    hpool = ctx.enter_context(tc.tile_pool(name="hbuf", bufs=1))
    gpool = ctx.enter_context(tc.tile_pool(name="gbuf", bufs=1))
    spool = ctx.enter_context(tc.tile_pool(name="small", bufs=4))
    ppool = ctx.enter_context(tc.tile_pool(name="psum", bufs=2, space="PSUM"))
    ptp = ctx.enter_context(tc.tile_pool(name="psumT", bufs=2, space="PSUM"))
    pzr = ctx.enter_context(tc.tile_pool(name="psumZR", bufs=1, space="PSUM"))
    pbc = ctx.enter_context(tc.tile_pool(name="psumBC", bufs=2, space="PSUM"))

    # ---- SP-ring bulk triggers (priority = emission order per ring).
    # NOTE: DMA-written tiles get TILE-granular read deps, so every
    # independently-consumed transfer gets its own tile. ----
    at_sb = []
    for g in range(GRAPH_NUM):
        t = cpool.tile([128, 4 * N], dt.bfloat16, name=f"adj{g}")
        nc.sync.dma_start(t[:], d_adj[g, :, :])
        at_sb.append(t)
    embrep_q = [cpool.tile([128, QD * N], dt.bfloat16, name=f"embrep{q}")
                for q in range(4)]
    WGT = NCH * O_G // 4
    wg_t = [cpool.tile([128, WGT], dt.bfloat16, name=f"wg{i}")
            for i in range(4)]
    sp_order = [(0, "e"), (1, "e"), (0, "w"), (2, "e"), (1, "w"), (3, "e"),
                (2, "w"), (3, "w")]
    for i, kind in sp_order:
        if kind == "e":
            nc.sync.dma_start(embrep_q[i][:], d_embrep[:, i * QD * N:(i + 1) * QD * N])
        else:
            nc.sync.dma_start(wg_t[i][:], d_wg[:, i * WGT:(i + 1) * WGT])

    # ---- ACT-ring: smallsA (hop-critical) first, then xsT, smallsB, wc ----
    SMALL_A = STATE2_C  # xsnat + ident
    smallsA = cpool.tile([128, SMALL_A], dt.bfloat16, name="smallsA")
    nc.scalar.dma_start(smallsA[:], d_smalls[:, 0:SMALL_A])
    smallsB = cpool.tile([128, SMALL_W - SMALL_A], dt.bfloat16, name="smallsB")
    xsnat_v = smallsA[:, XSNAT_C:XSNAT_C + 4 * IN_FEAT]
    ident_v = smallsA[:, IDENT_C:IDENT_C + 128]
    state2_v = smallsB[:, 0:N]
    embT_v = smallsB[0:D_EMB, EMBT_C - SMALL_A:EMBT_C - SMALL_A + N]
    bg_v = smallsB[0:D_EMB, BG_C - SMALL_A:BG_C - SMALL_A + O_G]
    bc_v = smallsB[0:D_EMB, BC_C - SMALL_A:BC_C - SMALL_A + O_C]
    xrow_v = smallsB[0:INPUT_DIM, XROW_C - SMALL_A:XROW_C - SMALL_A + N]
    WCT = NCH * O_C // 2
    wc_h = [cpool.tile([128, WCT], dt.bfloat16, name=f"wc{i}")
            for i in range(2)]

    # hT tiles + xs loads
    hT_g = [hpool.tile([128, N], dt.bfloat16, name=f"hTg{t}") for t in range(KCH)]
    hT_c = [hpool.tile([128, N], dt.bfloat16, name=f"hTc{t}") for t in range(KCH)]
    nc.vector.memset(hT_g[2][:, :], 0.0)
    nc.vector.memset(hT_c[2][:, :], 0.0)
    nc.scalar.dma_start(hT_g[0][0:IN_FEAT, :], d_xsT[:, :])
    # remaining ACT-ring bulk (after the hop-critical loads)
    nc.scalar.dma_start(smallsB[:], d_smalls[:, SMALL_A:SMALL_W])
    for t2 in range(2):
        nc.scalar.dma_start(wc_h[t2][:], d_wc[:, t2 * WCT:(t2 + 1) * WCT])

    # x rows into candidate hT tile0 (same partitions -> ACT copy, no DMA)
    nc.scalar.activation(hT_c[0][0:INPUT_DIM, :], xrow_v, Copy)

    # dummy matmuls warm the PE (HAM) while the adjacency streams in
    ones_sb = cpool.tile([128, 256], dt.bfloat16, name="ones_sb")
    nc.vector.memset(ones_sb[:, :], 1.0)
    for w in range(6):
        warm_ps = pbc.tile([128, 192], dt.float32, name=f"warm_ps{w}", tag="warmps", bufs=1)
        nc.tensor.matmul(warm_ps[:], ones_sb[:, 0:128], ones_sb[:, 0:192],
                         start=True, stop=True)

    # warm the ACT Copy table early (first pieceT copy needs it)
    warm = hpool.tile([1, 8], dt.float32, name="warm")
    nc.vector.memset(warm[:, :], 0.0)
    nc.scalar.activation(warm[:, 0:4], warm[:, 4:8], Copy)

    # gT buffer: 48 chunks of [128, N] in GROUP order (shared gate/cand)
    gT = gpool.tile([128, NCH * N], dt.bfloat16, name="gT")

    def piece_to_hT(hT, piece, piece_ps, p_idx, cand=False):
        """Place piece [IN_FEAT, N] into hT tiles. Split pieces (1 and 3)
        put their leading spill rows in the next tile via a base-0 ACT copy
        straight from PSUM (the host W-pack permutation compensates); the
        main part goes via a GpSimd SWDGE DMA (idle third ring). The
        candidate layout moves piece 1's main part to tile0[2:64]."""
        if p_idx == 1:
            nc.scalar.activation(hT[1][0:4, :], piece_ps[0:4, :], Copy)
            dst = hT[0][2:64, :] if cand else hT[0][66:128, :]
            return [nc.sync.dma_start(dst, piece[4:IN_FEAT, :])]
        if p_idx == 3:
            nc.scalar.activation(hT[2][0:8, :], piece_ps[0:8, :], Copy)
            return [nc.sync.dma_start(hT[1][70:128, :], piece[8:IN_FEAT, :])]
        r0 = IN_FEAT * p_idx
        t0, o0 = divmod(r0, 128)
        return [nc.sync.dma_start(hT[t0][o0:o0 + IN_FEAT, :], piece[:, :])]

    def hop(lhsT_of, g, name):
        """One propagation Y = A_g @ X, transposed out. lhsT_of(k)->AP [mlen,66]."""
        yt_ps = ppool.tile([IN_FEAT, N], dt.float32, name=f"ps_{name}", tag="hopps")
        for k, (moff, mlen) in enumerate(MCHUNKS):
            rhs = (at_sb[g][:, k * N:(k + 1) * N] if k < 3
                   else at_sb[g][0:16, 3 * N:4 * N])
            nc.tensor.matmul(
                yt_ps[:], lhsT_of(k), rhs,
                start=(k == 0), stop=(k == len(MCHUNKS) - 1),
            )
        yt = spool.tile([IN_FEAT, N], dt.bfloat16, name=f"yt_{name}", tag="hopsb")
        nc.scalar.activation(yt[:], yt_ps[:], Copy)
        return yt, yt_ps

    def nat_slicer(tl):
        return lambda k: tl[0:MCHUNKS[k][1], k * IN_FEAT:(k + 1) * IN_FEAT]

    def naturalize(yt, name):
        """PE-transpose YT [66, N] -> natural tile [128, 4*66]."""
        natt = spool.tile([128, 4 * IN_FEAT], dt.bfloat16, name=f"nat_{name}", tag="natsb")
        for k, (moff, mlen) in enumerate(MCHUNKS):
            tp = ptp.tile([mlen, IN_FEAT], dt.bfloat16, name=f"tp_{name}{k}", tag="trps")
            nc.tensor.transpose(tp[:], yt[:, moff:moff + mlen], ident_v[0:IN_FEAT, 0:IN_FEAT])
            nc.scalar.activation(natt[0:mlen, k * IN_FEAT:(k + 1) * IN_FEAT], tp[:], Copy)
        return natt

    filler_ctr = [100]

    def pe_fillers(n):
        for _ in range(n):
            warm_ps = pbc.tile([128, 192], dt.float32,
                               name=f"warm_ps{filler_ctr[0]}", tag="warmps", bufs=1)
            filler_ctr[0] += 1
            nc.tensor.matmul(warm_ps[:], ones_sb[:, 0:128], ones_sb[:, 0:192],
                             start=True, stop=True)

    def meta_phase(hT, lhsT_of, w_of, b_sb, o_dim, psum_out, phase, cand=False):
        """Hops + gT build + meta matmul, accumulating into psum_out [o_dim, N]."""
        # both first hops are independent: run them (and their pieces) first
        y1 = []
        for g in range(GRAPH_NUM):
            y1t, y1ps = hop(lhsT_of, g, f"{phase}y1g{g}")
            piece_to_hT(hT, y1t, y1ps, 1 + 2 * g, cand=cand)
            y1.append(y1t)
        y1nat = [naturalize(y1[g], f"{phase}g{g}") for g in range(GRAPH_NUM)]
        for g in range(GRAPH_NUM):
            y2t, y2ps = hop(nat_slicer(y1nat[g]), g, f"{phase}y2g{g}")
            piece_to_hT(hT, y2t, y2ps, 2 + 2 * g, cand=cand)

        if not cand:
            # load sigma/tanh ACT tables in the phase's ACT slack window
            nc.scalar.activation(warm[:, 0:4], warm[:, 4:8], Sig)
            nc.scalar.activation(warm[:, 0:4], warm[:, 4:8], Tanh)
        # bias matmul resets PSUM
        nc.tensor.matmul(psum_out[:], b_sb[:], embT_v, start=True, stop=False)

        # gT build (fused 4-d DVE ops) + accumulate matmuls; k-outer order
        for gi, (k, q) in enumerate(GROUPS):
            out_ap = (gT[:, gi * QD * N:(gi + 1) * QD * N]
                      .rearrange("p (c n) -> p c n", n=N))
            in0 = (hT[k][:, :].rearrange("p (u n) -> p u n", u=1)
                   .broadcast_to([128, QD, N]))
            in1 = embrep_q[q][:, :].rearrange("p (c n) -> p c n", n=N)
            nc.vector.tensor_tensor(out_ap, in0, in1, mybir.AluOpType.mult)
            for j in range(QD):
                c = gi * QD + j
                nc.tensor.matmul(
                    psum_out[:],
                    w_of(c),
                    gT[:, c * N:(c + 1) * N],
                    start=False,
                    stop=(gi == len(GROUPS) - 1 and j == QD - 1),
                )

    def wg_of(c):
        return wg_t[c // 12][:, (c % 12) * O_G:(c % 12 + 1) * O_G]

    def wc_of(c):
        return wc_h[c // 24][:, (c % 24) * O_C:(c % 24 + 1) * O_C]

    # ================= gate phase =================
    zr_ps = pzr.tile([O_G, N], dt.float32, name="zr_ps")
    meta_phase(hT_g, nat_slicer(xsnat_v), wg_of, bg_v, O_G, zr_ps, "g")
    zr_sig = hpool.tile([O_G, N], dt.float32, name="zr_sig")
    # r-half first so the candidate chain starts as early as possible
    nc.scalar.activation(zr_sig[HIDDEN:O_G, :], zr_ps[HIDDEN:O_G, :], Sig)
    nc.scalar.activation(zr_sig[0:HIDDEN, :], zr_ps[0:HIDDEN, :], Sig)

    # rs written straight into the candidate hT tile (base 64, no shift DMA);
    # the Wc host packing uses the matching i-permutation
    nc.vector.tensor_mul(hT_c[0][HIDDEN:O_G, :], zr_sig[HIDDEN:O_G, :],
                         state2_v[HIDDEN:O_G, :])

    # keep the PE busy across the sigma/rs transition
    pe_fillers(6)
    # xrs natural from the two aligned regions: x rows 0:2, rs rows 64:128
    xrsnat = spool.tile([128, 4 * IN_FEAT], dt.bfloat16, name="nat_xrs", tag="natsb")
    for k, (moff, mlen) in enumerate(MCHUNKS):
        tpx = ptp.tile([mlen, INPUT_DIM], dt.bfloat16, name=f"tpx{k}", tag="trpsx", bufs=1)
        nc.tensor.transpose(tpx[:], hT_c[0][0:INPUT_DIM, moff:moff + mlen],
                            ident_v[0:INPUT_DIM, 0:INPUT_DIM])
        nc.scalar.activation(
            xrsnat[0:mlen, k * IN_FEAT:k * IN_FEAT + INPUT_DIM], tpx[:], Copy)
        tpr = ptp.tile([mlen, HIDDEN], dt.bfloat16, name=f"tpr{k}", tag="trps")
        nc.tensor.transpose(tpr[:], hT_c[0][HIDDEN:O_G, moff:moff + mlen],
                            ident_v[HIDDEN:O_G, HIDDEN:O_G])
        nc.scalar.activation(
            xrsnat[0:mlen, k * IN_FEAT + INPUT_DIM:(k + 1) * IN_FEAT], tpr[:], Copy)

    # ================= candidate phase =================
    hc_ps = pzr.tile([O_C, N], dt.float32, name="hc_ps")
    meta_phase(hT_c, nat_slicer(xrsnat), wc_of, bc_v, O_C, hc_ps, "c", cand=True)

    # z-dependent blend terms precomputed while the candidate phase runs:
    # h = hc + z*(state - hc) = (1-z)*hc + z*state
    omz = hpool.tile([O_C, N], dt.float32, name="omz")
    nc.vector.tensor_scalar(omz[:], zr_sig[0:HIDDEN, :], -1.0, 1.0,
                            mybir.AluOpType.mult, mybir.AluOpType.add)
    zs = hpool.tile([O_C, N], dt.float32, name="zs")
    nc.vector.tensor_mul(zs[:], zr_sig[0:HIDDEN, :], state2_v[0:HIDDEN, :])

    hc_t = hpool.tile([O_C, N], dt.float32, name="hc_t")
    nc.scalar.activation(hc_t[:], hc_ps[:], Tanh)

    # ================= output blend =================
    d2 = hpool.tile([O_C, N], dt.float32, name="d2")
    nc.vector.tensor_mul(d2[:], omz[:], hc_t[:])
    hout = hpool.tile([O_C, N], dt.float32, name="hout")
    nc.vector.tensor_add(hout[:], d2[:], zs[:])
    nc.sync.dma_start(d_out[:, :], hout[:])


def _build_nc():
    import concourse.tile as tile
    import concourse.mybir as mybir
    from contextlib import ExitStack
    from concourse import bacc

    nc = bacc.Bacc(trn_type="TRN2")
    with tile.TileContext(nc) as tc:
        with ExitStack() as ctx:
            _emit(nc, tc, tile, mybir, ctx)
    nc.finalize()
    return nc


def _prep_core_inputs(b, x, state, graphs, node_emb, Wg, bg, Wc, bc):
    """Host-side shard + layout prep for core b. Layouts match SBUF tiles."""
    f32 = np.float32
    at = graphs[:, b].transpose(0, 2, 1)                         # [G, N, N] = A.T
    at_pk = (at[:, :384, :].reshape(GRAPH_NUM, 3, 128, N)
             .transpose(0, 2, 1, 3)
             .reshape(GRAPH_NUM, 128, 3 * N))                    # [G,128,(k n)]
    at3 = at[:, 384:400, :]                                      # [G,16,N]
    adj = np.zeros((GRAPH_NUM, 128, 4 * N), f32)
    adj[:, :, :3 * N] = at_pk
    adj[:, 0:16, 3 * N:] = at3

    xs = np.concatenate([x[b], state[b]], axis=-1)               # [N, 66] f32
    xsT = np.ascontiguousarray(xs.T).astype(BF16)                # [66, N]
    xs_pad = np.zeros((NPAD, IN_FEAT), f32)
    xs_pad[:N] = xs
    xsnat = (xs_pad.reshape(4, 128, IN_FEAT)
             .transpose(1, 0, 2)
             .reshape(128, 4 * IN_FEAT))                         # [128,(k f)]
    stT = np.ascontiguousarray(state[b].T)                       # [64, N] f32
    embT = np.ascontiguousarray(node_emb[b].T).astype(BF16)      # [16, N]
    embrep = np.ascontiguousarray(np.broadcast_to(
        embT.reshape(1, D_EMB * N), (128, D_EMB * N)))           # [128, 16N]

    smalls = np.zeros((128, SMALL_W), f32)
    smalls[:, XSNAT_C:XSNAT_C + 4 * IN_FEAT] = xsnat
    smalls[:, IDENT_C:IDENT_C + 128] = np.eye(128, dtype=f32)
    smalls[0:HIDDEN, STATE2_C:STATE2_C + N] = stT
    smalls[HIDDEN:O_G, STATE2_C:STATE2_C + N] = stT
    smalls[0:D_EMB, EMBT_C:EMBT_C + N] = embT.astype(f32)
    smalls[0:D_EMB, BG_C:BG_C + O_G] = bg
    smalls[0:D_EMB, BC_C:BC_C + O_C] = bc
    smalls[0:INPUT_DIM, XROW_C:XROW_C + N] = x[b].T

    def pack_w(W, o_dim, perm):
        # W [16, 330, o] -> [128, 48*o] in GROUP chunk order; chunk (d,k):
        # padded row r=128k+p holds reference feature perm[r]
        Wp = np.zeros((D_EMB, I_PAD, o_dim), np.float32)
        valid = perm >= 0
        Wp[:, valid, :] = W[:, perm[valid], :]
        Wp = Wp.reshape(D_EMB, KCH, 128, o_dim)                  # [d,k,p,o]
        cols = np.empty((128, NCH * o_dim), np.float32)
        ci = 0
        for (k, q) in GROUPS:
            for j in range(QD):
                d = q * QD + j
                cols[:, ci * o_dim:(ci + 1) * o_dim] = Wp[d, k]
                ci += 1
        return np.ascontiguousarray(cols).astype(BF16)

    # spill permutation (both phases): pieces 1/3 put their first 4/8 rows
    # in the next tile, so main parts shift by the spill size
    perm_g = np.arange(I_PAD, dtype=np.int64)
    perm_g[I_DIM:] = -1
    perm_g[66:128] = np.arange(70, 132)
    perm_g[128:132] = np.arange(66, 70)
    perm_g[198:256] = np.arange(206, 264)
    perm_g[256:264] = np.arange(198, 206)
    # candidate adds: rows 2:64 <- Y1g0 main (ref 70:132), rows 64:128 <- rs
    perm_c = perm_g.copy()
    perm_c[0:INPUT_DIM] = np.arange(0, INPUT_DIM)
    perm_c[2:64] = np.arange(70, 132)
    perm_c[64:128] = np.arange(2, 66)
    perm_c[128:132] = np.arange(66, 70)

    return {
        "adj": np.ascontiguousarray(adj).astype(BF16),
        "smalls": np.ascontiguousarray(smalls).astype(BF16),
        "xsT": xsT,
        "embrep": embrep,
        "wg": pack_w(Wg, O_G, perm_g),
        "wc": pack_w(Wc, O_C, perm_c),
    }


def kernel_with_results(x, state, graphs, node_emb, Wg, bg, Wc, bc, trace=False):
    from concourse.bass_utils import run_bass_kernel_spmd

    x = np.asarray(x, np.float32)
    state = np.asarray(state, np.float32)
    graphs = np.asarray(graphs, np.float32)
    node_emb = np.asarray(node_emb, np.float32)
    Wg = np.asarray(Wg, np.float32)
    bg = np.asarray(bg, np.float32)
    Wc = np.asarray(Wc, np.float32)
    bc = np.asarray(bc, np.float32)

    if "nc" not in _CACHE:
        _CACHE["nc"] = _build_nc()
    nc = _CACHE["nc"]

    in_maps = [
        _prep_core_inputs(b, x, state, graphs, node_emb, Wg, bg, Wc, bc)
        for b in range(B)
    ]
    res = run_bass_kernel_spmd(nc, in_maps, core_ids=list(range(B)), trace=trace)
    out = np.stack(
        [np.ascontiguousarray(res.results[b]["out"].T) for b in range(B)], axis=0
    )  # [B, N, HIDDEN] f32
    return out, res


def kernel(**inputs):
    out, _ = kernel_with_results(**inputs)
    return out
